# revision 1
# baseline (speedup 1.0000x reference)
"""DiagonalLSTM Trainium2 kernel.

Sharding: data-parallel over batch B=16 across 8 cores (2 batch elems/core).
Per-core layout: partitions = 128-wide HID gate chunks, free dim = (b, h).

Per scan step t (127 steps), each of the 5 gate chunks accumulates in PSUM:
    wis_aug @ x_diag   (K=65: 64 channels + ones row folding b_is+b_ss)
  + w0_chunk @ h_prev  written column-shifted by one H position
  + w1_chunk @ h_prev
All matmuls fp32: the scan dynamics chaotically amplify per-step rounding
noise, so bf16/fp16 inputs fail; fp32 matches the fp32-reference envelope.

Sigmoid gates computed as 0.5*(1+tanh(x/2)) — ACT tanh is ~2 ULP vs ~20 ULP
native sigmoid, which matters under the chaotic amplification. The sigmoid
chunks' weights/biases are pre-halved on the host so ONE tanh activation over
all 5 gate chunks serves both the 4 sigmoid gates and the g-gate.

The x-side matmuls for step t+1 are emitted right after step t's tap matmuls
so the PE stays busy while ACT/DVE run step t's nonlinear/elementwise chain.
h is accumulated into a residual tile along stride-63 diagonal APs; one DMA
out at the end.
"""

import numpy as np

import concourse.bass as bass
import concourse.mybir as mybir
from concourse import bacc
from concourse import tile
from concourse.bass_utils import run_bass_kernel_spmd

B, C, H, W = 16, 64, 64, 64
HID = 128
SW = H + W - 1  # 127
NCORES = 8
BL = B // NCORES  # 2
NBH = BL * H      # 128 free columns (b, h)
NXC = BL * H * SW  # 16256 skewed cols
NRES = BL * H * W  # 8192 output cols

F32 = mybir.dt.float32
AF = mybir.ActivationFunctionType
ALU = mybir.AluOpType

# Use single matmuls with 2D free APs spanning both batch blocks (fewer
# instructions and, more importantly, fewer LDWEIGHTS on hardware).  The
# executing simulator can't evaluate 2D-free matmuls (shape assert), so
# dbg_sim sets this False and rebuilds for numerics validation — the two
# forms are mechanically equivalent.
B2D = True


def _raw(t, off, dims):
    """Raw AP on tile t: keep its partition pair, custom free dims."""
    return bass.AP(t.tensor, t.offset + off, [list(t.ap[0])] + [list(d) for d in dims])


def build_program():
    nc = bacc.Bacc(None, target_bir_lowering=False)

    xsk_d = nc.dram_tensor("xsk", [C, NXC], F32, kind="ExternalInput")
    xres_d = nc.dram_tensor("xres", [C + 1, NRES], F32, kind="ExternalInput")
    wtap_d = nc.dram_tensor("wtap", [HID, 2 * 5 * HID], F32, kind="ExternalInput")
    wis_d = nc.dram_tensor("wis", [C, 5 * HID], F32, kind="ExternalInput")
    wres_d = nc.dram_tensor("wres", [C + 1, HID], F32, kind="ExternalInput")
    bias_d = nc.dram_tensor("bias", [HID, 5], F32, kind="ExternalInput")
    out_d = nc.dram_tensor("out", [HID, NRES], F32, kind="ExternalOutput")

    with tile.TileContext(nc) as tc:
        with (
            tc.tile_pool(name="const", bufs=1) as const,
            tc.tile_pool(name="state", bufs=3) as state,
            tc.tile_pool(name="tmp", bufs=3) as tmp,
            tc.tile_pool(name="gpsum", bufs=8, space="PSUM") as gpsum,
        ):
            xsk = const.tile([C, NXC], F32)
            xres = const.tile([C + 1, NRES], F32)
            wtap = const.tile([HID, 2 * 5 * HID], F32)
            wis = const.tile([C, 5 * HID], F32)
            wres = const.tile([C + 1, HID], F32)
            bias = const.tile([HID, 5], F32)
            res = const.tile([HID, NRES], F32)

            # weights on the sync queue; xsk t-major ([c, t, b, r]) in
            # chunks on the gpsimd queue (parallel with the weights) so the
            # scan starts as soon as the first steps' columns land; xres
            # (needed from step 70) last.
            nc.sync.dma_start(out=wis, in_=wis_d[:])
            nc.sync.dma_start(out=bias, in_=bias_d[:])
            nc.sync.dma_start(out=wtap, in_=wtap_d[:])
            nc.sync.dma_start(out=wres, in_=wres_d[:])
            cuts = [0, 4 * NBH, 16 * NBH] + list(
                range(32 * NBH, NXC, 16 * NBH)
            ) + [NXC]
            for lo, hi in zip(cuts[:-1], cuts[1:]):
                nc.sync.dma_start(out=xsk[:, lo:hi], in_=xsk_d[:, lo:hi])
            nc.sync.dma_start(out=xres, in_=xres_d[:])

            # Each PSUM accumulation group gets its own one-bank pool tile;
            # the 8-deep pool recycles banks round-robin, so at most one
            # open group per bank (HW zero-region constraint) and next-step
            # x-matmuls open groups in banks whose previous group was
            # consumed ~1.6 steps earlier.
            def pbank():
                ps = gpsum.tile([HID, 512], F32, tag="ps")
                return ps

            # ---- scan state ----
            h_cur = state.tile([HID, NBH], F32, tag="h")
            c_cur = state.tile([HID, NBH], F32, tag="c")
            nc.vector.memzero(h_cur)
            nc.vector.memzero(c_cur)

            # scan chunk emission order (gate chunk index k): fl, fu, i, g, o
            KORD = (1, 2, 3, 4, 0)

            def xmm(t):
                """i_s matmuls for step t (opens the 5 groups).

                Only the x-valid window [max(0,t-63) .. min(t,63)] is
                computed: below it the skew is zero (i_s contribution = 0,
                bias arrives via the ACT bias port), above it rows are dead
                (t >= W).  PSUM cols not written here are zeroed by the tap
                matmuls (pending-zero semantics).
                """
                vlo = max(0, t - (W - 1))
                base = t * NBH
                tiles = []
                for k in KORD:
                    pk = pbank()[:, 0:NBH]
                    wc = wis[:, k * HID:(k + 1) * HID]
                    if vlo == 0:
                        nc.tensor.matmul(
                            pk, wc, xsk[:, base:base + NBH],
                            start=True, stop=False,
                        )
                    elif B2D:
                        pkv = pk.rearrange("p (b r) -> p b r", b=BL)[:, :, vlo:]
                        xv = xsk[:, base:base + NBH].rearrange(
                            "p (b r) -> p b r", b=BL
                        )[:, :, vlo:]
                        nc.tensor.matmul(pkv, wc, xv, start=True, stop=False)
                    else:
                        for b in range(BL):
                            nc.tensor.matmul(
                                pk[:, b * H + vlo:(b + 1) * H],
                                wc, xsk[:, base + b * H + vlo:base + (b + 1) * H],
                                start=(b == 0), stop=False,
                            )
                    tiles.append(pk)
                return tiles

            pcur = xmm(0)

            # State convention: h_cur holds 2h, c_cur holds 2c.  Sigmoid
            # gates are T/2 with T = tanh(z/2)+1 (weights pre-halved on the
            # host; tap weights additionally halved to absorb the 2h).
            #   C2'   = 0.5*(T_fl*C2 + T_fu*C2sh) + T_i*g
            #   H2'   = T_o * tanh(C2'/2)
            #   res  += 0.5*H2'
            # Tap matmuls are emitted in chunk order fl,fu,i,g,o so the gate
            # tanh can run in three slices overlapping the remaining taps.
            for t in range(SW):
                lo = max(0, t - (W - 1))

                def V(ap, a=None):
                    """Live-range view [lo..H) of each batch block (3D)."""
                    s = lo if a is None else a
                    return ap.rearrange("p (b r) -> p b r", b=BL)[:, :, s:H]

                th = tmp.tile([HID, 5 * HID], F32, tag="th")
                for idx, k in enumerate(KORD):
                    pk = pcur[idx]
                    w0c = wtap[:, k * HID:(k + 1) * HID]
                    w1c = wtap[:, 5 * HID + k * HID: 5 * HID + (k + 1) * HID]
                    # w0 @ h_prev, H-shifted, live rows only
                    s0 = max(lo, 1)
                    if B2D:
                        nc.tensor.matmul(
                            V(pk, s0), w0c,
                            h_cur.rearrange("p (b r) -> p b r", b=BL)[:, :, s0 - 1:H - 1],
                            start=False, stop=False,
                        )
                    else:
                        for b in range(BL):
                            nc.tensor.matmul(
                                pk[:, b * H + s0:(b + 1) * H],
                                w0c, h_cur[:, b * H + s0 - 1:(b + 1) * H - 1],
                                start=False, stop=False,
                            )
                    # w1 @ h_prev
                    if lo == 0:
                        nc.tensor.matmul(pk, w1c, h_cur, start=False, stop=True)
                    elif B2D:
                        nc.tensor.matmul(
                            V(pk), w1c, V(h_cur), start=False, stop=True
                        )
                    else:
                        for b in range(BL):
                            nc.tensor.matmul(
                                pk[:, b * H + lo:(b + 1) * H],
                                w1c, h_cur[:, b * H + lo:(b + 1) * H],
                                start=False, stop=(b == BL - 1),
                            )
                    # per-chunk tanh (+ per-partition gate bias): fires as
                    # soon as this chunk's bank is complete, overlapping
                    # the remaining taps
                    thc = th[:, idx * HID:(idx + 1) * HID]
                    bk = bias[:, k:k + 1]
                    if lo == 0:
                        nc.scalar.activation(thc, pk, AF.Tanh, bias=bk)
                    else:
                        nc.scalar.activation(V(thc), V(pk), AF.Tanh, bias=bk)

                # prefetch next step's x-side matmuls while ACT/DVE run
                if t + 1 < SW:
                    pcur = xmm(t + 1)

                t_fl = th[:, 0:HID]
                t_fu = th[:, HID:2 * HID]
                t_i = th[:, 2 * HID:3 * HID]
                g = th[:, 3 * HID:4 * HID]
                t_o = th[:, 4 * HID:5 * HID]

                # P = (t_fl+1)*C2 ; P += (t_fu+1)*C2sh (rows >= max(lo,1));
                # C2' = 0.5*P + (t_i+1)*g   — all on live rows [lo..H)
                p = tmp.tile([HID, NBH], F32, tag="p")
                nc.vector.scalar_tensor_tensor(
                    V(p), V(t_fl), 1.0, V(c_cur), op0=ALU.add, op1=ALU.mult
                )
                q = tmp.tile([HID, NBH], F32, tag="q")
                s0 = max(lo, 1)
                cc_sh = c_cur.rearrange("p (b r) -> p b r", b=BL)[:, :, s0 - 1:H - 1]
                nc.vector.scalar_tensor_tensor(
                    V(q, s0), V(t_fu, s0), 1.0, cc_sh,
                    op0=ALU.add, op1=ALU.mult,
                )
                nc.vector.tensor_add(V(p, s0), V(p, s0), V(q, s0))
                r_t = tmp.tile([HID, NBH], F32, tag="r_t")
                nc.vector.scalar_tensor_tensor(
                    V(r_t), V(t_i), 1.0, V(g), op0=ALU.add, op1=ALU.mult
                )
                c_new = state.tile([HID, NBH], F32, tag="c")
                nc.vector.scalar_tensor_tensor(
                    V(c_new), V(p), 0.5, V(r_t), op0=ALU.mult, op1=ALU.add
                )

                tanc = tmp.tile([HID, NBH], F32, tag="tanc")
                nc.scalar.activation(V(tanc), V(c_new), AF.Tanh, scale=0.5)
                h_new = state.tile([HID, NBH], F32, tag="h")
                nc.vector.scalar_tensor_tensor(
                    V(h_new), V(t_o), 1.0, V(tanc), op0=ALU.add, op1=ALU.mult
                )

                # write H2 into res along the diagonal w = t - r (gpsimd:
                # keeps DVE free).  Each res cell is touched exactly once,
                # so this is a copy — no init needed.  res holds
                # 2*(residual + h); the host halves the output.
                rlo = max(0, t - (W - 1))
                rhi = min(H - 1, t)
                nr = rhi - rlo + 1
                res_ap = _raw(
                    res, (W - 1) * rlo + t, [[H * W, BL], [W - 1, nr]]
                )
                h_ap = h_new.rearrange("p (b r) -> p b r", b=BL)[:, :, rlo:rhi + 1]
                nc.gpsimd.tensor_copy(out=res_ap, in_=h_ap)

                h_cur = h_new
                c_cur = c_new

                # Late-scan interleave: once an 8-row block's diagonal cells
                # are all written (t = 8j+70), add its residual
                # (w_res @ x + b_res, doubled on host) and DMA it out.
                # These fill the PE/DVE idle left by the shrinking tail.
                if t >= 70 and (t - 70) % 8 == 0 and (t - 70) // 8 < 8:
                    j = (t - 70) // 8
                    for b in range(BL):
                        cols = slice(b * H * W + 512 * j, b * H * W + 512 * j + 512)
                        rp = pbank()
                        nc.tensor.matmul(
                            rp, wres, xres[:, cols], start=True, stop=True
                        )
                        nc.vector.tensor_add(res[:, cols], res[:, cols], rp)
                        nc.sync.dma_start(out=out_d[:, cols], in_=res[:, cols])

    nc.finalize()
    return nc


_NC_CACHE = {}


def _get_nc():
    if "nc" not in _NC_CACHE:
        _NC_CACHE["nc"] = build_program()
    return _NC_CACHE["nc"]


def _prep_inputs(x, w_is, b_is, w_ss, b_ss, w_res, b_res):
    x = np.asarray(x, np.float32)
    # skewed x: [B, C, H, SW], row r shifted right by r
    sk = np.zeros((B, C, H, SW), np.float32)
    for r in range(H):
        sk[:, :, r, r:r + W] = x[:, :, r, :]
    # [core, c, t, b, r] (t-major so each step reads a contiguous slice)
    xsk = sk.reshape(NCORES, BL, C, H, SW).transpose(0, 2, 4, 1, 3)
    xsk = np.ascontiguousarray(xsk).reshape(NCORES, C, NXC)

    xres = np.asarray(x).reshape(NCORES, BL, C, H, W).transpose(0, 2, 1, 3, 4)
    xres = xres.reshape(NCORES, C, NRES)
    xres = np.concatenate([xres, np.ones((NCORES, 1, NRES), np.float32)], axis=1)

    # gate scaling: chunks 0..3 (o, f_left, f_up, i) are sigmoid gates,
    # computed via tanh(z/2) -> pre-halve their weights and biases.
    gs = np.ones((5 * HID,), np.float32)
    gs[0:4 * HID] = 0.5

    # wtap[i, tap*640 + o] = w_ss[o, i, tap] * gs[o] * 0.5
    # (extra 0.5: the kernel's h state holds 2h)
    wtap = np.asarray(w_ss, np.float32).transpose(1, 2, 0) * (0.5 * gs)[None, None, :]
    wtap = np.ascontiguousarray(wtap.reshape(HID, 2 * 5 * HID), np.float32)
    wis = np.ascontiguousarray(
        np.asarray(w_is, np.float32).T * gs[None, :], np.float32
    )
    # per-partition gate bias, fed through the ACT bias port: [128, 5]
    bvec = (np.asarray(b_is, np.float32) + np.asarray(b_ss, np.float32)) * gs
    biases = np.ascontiguousarray(bvec.reshape(5, HID).T, np.float32)
    # x2: the device residual tile accumulates 2*(residual + sum h); the
    # host halves the final output.
    wres = 2.0 * np.concatenate(
        [np.asarray(w_res, np.float32).T, np.asarray(b_res, np.float32)[None, :]],
        axis=0,
    ).astype(np.float32)

    in_maps = []
    for c in range(NCORES):
        in_maps.append({
            "xsk": np.ascontiguousarray(xsk[c]),
            "xres": np.ascontiguousarray(xres[c]),
            "wtap": wtap,
            "wis": wis,
            "wres": wres,
            "bias": biases,
        })
    return in_maps


def kernel(x, w_is, b_is, w_ss, b_ss, w_res, b_res, _trace=False):
    nc = _get_nc()
    in_maps = _prep_inputs(x, w_is, b_is, w_ss, b_ss, w_res, b_res)
    r = run_bass_kernel_spmd(nc, in_maps, list(range(NCORES)), trace=_trace)
    outs = [r.results[c]["out"] for c in range(NCORES)]
    out = np.stack(outs, 0).reshape(NCORES, HID, BL, H, W)
    out = out.transpose(0, 2, 1, 3, 4).reshape(B, HID, H, W)
    return np.ascontiguousarray(out * np.float32(0.5))



# revision 3
# speedup vs baseline: 1.2370x; 1.2370x over previous
"""DiagonalLSTM Trainium2 kernel — band-restricted scan.

Sharding: data-parallel over batch B=16 across 8 cores (2 batch elems/core).
Per-core layout: partitions = 128-wide HID gate chunks, free dim = (b, j)
where j indexes the LIVE DIAGONAL BAND rows [lo..hi], lo = max(0, t-63),
hi = min(t, 63).

Key reduction vs the full-width scan: rows r > t ("virgin" rows, zero x so
far) all share one state vector v_t that depends only on t, so they are not
computed on-device at all.  A host-precomputed fp64 table of v_t (h and c,
device 2x convention) seeds row t+1 each step via two 1-col gpsimd copies.
Per-step matmul/ACT/DVE free size drops from avg 96 to avg 64.5 columns and
the x-side input is pre-packed band-only (xsk [64, 8192] vs [64, 16256]).

Per scan step t (127 steps), each of the 5 gate chunks accumulates in PSUM:
    wis_chunk @ x_band   (K=64, packed band cols, opens the group)
  + w0_chunk @ h_prev    row-shifted (skipped at t=0)
  + w1_chunk @ h_prev
All scan matmuls fp32: the scan dynamics chaotically amplify per-step input
rounding (measured: fp32r inputs -> rel err 1.4), so only the feed-forward
residual matmul uses fp32r (4x faster, error enters once, ~1e-4).

Sigmoid gates computed as 0.5*(1+tanh(x/2)) via pre-halved weights; ONE tanh
activation per gate chunk (fires as soon as its PSUM bank closes, keeping the
serial chain short).  State convention: h_cur holds 2h, c_cur holds 2c; res
accumulates 2*(h+residual) and the host halves the output.
"""

import numpy as np

import concourse.bass as bass
import concourse.mybir as mybir
from concourse import bacc
from concourse import tile
from concourse.bass_utils import run_bass_kernel_spmd

B, C, H, W = 16, 64, 64, 64
HID = 128
SW = H + W - 1  # 127
NCORES = 8
BL = B // NCORES  # 2
NBH = BL * H       # 128 state cols (b, r)
NRES = BL * H * W  # 8192 output cols

F32 = mybir.dt.float32
F32R = mybir.dt.float32r
AF = mybir.ActivationFunctionType
ALU = mybir.AluOpType

# band geometry per step (shared host/device)
_LO = [max(0, t - (W - 1)) for t in range(SW)]
_HI = [min(t, H - 1) for t in range(SW)]
_M = [hi - lo + 1 for lo, hi in zip(_LO, _HI)]
_BASE = np.concatenate([[0], np.cumsum([BL * m for m in _M])]).astype(int)
XC = int(_BASE[-1])  # 8192

# scan chunk emission order (gate chunk index k): fl, fu, i, g, o
KORD = (1, 2, 3, 4, 0)


def _raw(t, off, dims):
    """Raw AP on tile t: keep its partition pair, custom free dims."""
    return bass.AP(t.tensor, t.offset + off, [list(t.ap[0])] + [list(d) for d in dims])


def build_program():
    nc = bacc.Bacc(None, target_bir_lowering=False)

    xsk_d = nc.dram_tensor("xsk", [C, XC], F32, kind="ExternalInput")
    xres_d = nc.dram_tensor("xres", [C + 1, NRES], F32R, kind="ExternalInput")
    wtap_d = nc.dram_tensor("wtap", [HID, 2 * 5 * HID], F32, kind="ExternalInput")
    wis_d = nc.dram_tensor("wis", [C, 5 * HID], F32, kind="ExternalInput")
    wres_d = nc.dram_tensor("wres", [C + 1, HID], F32R, kind="ExternalInput")
    bias_d = nc.dram_tensor("bias", [HID, 5], F32, kind="ExternalInput")
    hv_d = nc.dram_tensor("hv", [HID, H - 1], F32, kind="ExternalInput")
    cv_d = nc.dram_tensor("cv", [HID, H - 1], F32, kind="ExternalInput")
    out_d = nc.dram_tensor("out", [HID, NRES], F32, kind="ExternalOutput")

    with tile.TileContext(nc) as tc:
        with (
            tc.tile_pool(name="const", bufs=1) as const,
            tc.tile_pool(name="state", bufs=3) as state,
            tc.tile_pool(name="tmp", bufs=3) as tmp,
            tc.tile_pool(name="gpsum", bufs=8, space="PSUM") as gpsum,
        ):
            xsk = const.tile([C, XC], F32)
            xres = const.tile([C + 1, NRES], F32R)
            wtap = const.tile([HID, 2 * 5 * HID], F32)
            wis = const.tile([C, 5 * HID], F32)
            wres = const.tile([C + 1, HID], F32R)
            bias = const.tile([HID, 5], F32)
            hv = const.tile([HID, H - 1], F32)
            cv = const.tile([HID, H - 1], F32)
            res = const.tile([HID, NRES], F32)

            nc.sync.dma_start(out=wis, in_=wis_d[:])
            nc.sync.dma_start(out=bias, in_=bias_d[:])
            nc.sync.dma_start(out=wtap, in_=wtap_d[:])
            nc.sync.dma_start(out=hv, in_=hv_d[:])
            nc.sync.dma_start(out=cv, in_=cv_d[:])
            nc.sync.dma_start(out=wres, in_=wres_d[:])
            steps_cut = [0, 4, 12, 24, 36, 48, 64, 80, 100, SW]
            for a, b in zip(steps_cut[:-1], steps_cut[1:]):
                lo_e, hi_e = int(_BASE[a]), int(_BASE[b])
                nc.sync.dma_start(out=xsk[:, lo_e:hi_e], in_=xsk_d[:, lo_e:hi_e])
            nc.sync.dma_start(out=xres, in_=xres_d[:])

            def pbank():
                ps = gpsum.tile([HID, 512], F32, tag="ps")
                return ps

            # ---- scan state: (b, r) layout [HID, 128] ----
            h_cur = state.tile([HID, NBH], F32, tag="h")
            c_cur = state.tile([HID, NBH], F32, tag="c")
            nc.vector.memzero(h_cur)
            nc.vector.memzero(c_cur)

            def B3(ap, a, b):
                """(b, r) state view, rows [a..b) of each batch block."""
                return ap.rearrange("p (b r) -> p b r", b=BL)[:, :, a:b]

            def xmm(t):
                """i_s matmuls for step t: packed band cols (opens groups)."""
                b0, n = int(_BASE[t]), BL * _M[t]
                tiles = []
                for k in KORD:
                    pk = pbank()
                    nc.tensor.matmul(
                        _raw(pk, 0, [[1, n]]),
                        wis[:, k * HID:(k + 1) * HID],
                        xsk[:, b0:b0 + n],
                        start=True, stop=False,
                    )
                    tiles.append(pk)
                return tiles

            pcur = xmm(0)

            for t in range(SW):
                lo, hi, m = _LO[t], _HI[t], _M[t]
                n = BL * m
                s0 = max(lo, 1)
                mq = hi - s0 + 1  # rows with a defined (r-1) neighbor

                th = tmp.tile([HID, 5 * HID], F32, tag="th")
                for idx, k in enumerate(KORD):
                    pk = pcur[idx]
                    w0c = wtap[:, k * HID:(k + 1) * HID]
                    w1c = wtap[:, 5 * HID + k * HID:5 * HID + (k + 1) * HID]
                    if mq > 0:
                        nc.tensor.matmul(
                            _raw(pk, s0 - lo, [[m, BL], [1, mq]]),
                            w0c,
                            B3(h_cur, s0 - 1, hi),
                            start=False, stop=False,
                        )
                    nc.tensor.matmul(
                        _raw(pk, 0, [[m, BL], [1, m]]),
                        w1c,
                        B3(h_cur, lo, hi + 1),
                        start=False, stop=True,
                    )
                    # per-chunk tanh (+ per-partition gate bias): fires as
                    # soon as this chunk's bank closes
                    nc.scalar.activation(
                        _raw(th, idx * HID, [[1, n]]),
                        _raw(pk, 0, [[1, n]]),
                        AF.Tanh, bias=bias[:, k:k + 1],
                    )

                # prefetch next step's x-side matmuls while ACT/DVE run
                if t + 1 < SW:
                    pcur = xmm(t + 1)

                # P = (t_fl+1)*C2 ; P += (t_fu+1)*C2sh (rows >= s0);
                # C2' = 0.5*P + (t_i+1)*g  on band rows
                p = tmp.tile([HID, NBH], F32, tag="p")
                nc.vector.scalar_tensor_tensor(
                    _raw(p, 0, [[m, BL], [1, m]]),
                    _raw(th, 0 * HID, [[m, BL], [1, m]]),
                    1.0, B3(c_cur, lo, hi + 1), op0=ALU.add, op1=ALU.mult,
                )
                if mq > 0:
                    q = tmp.tile([HID, NBH], F32, tag="q")
                    nc.vector.scalar_tensor_tensor(
                        _raw(q, s0 - lo, [[m, BL], [1, mq]]),
                        _raw(th, 1 * HID + (s0 - lo), [[m, BL], [1, mq]]),
                        1.0, B3(c_cur, s0 - 1, hi), op0=ALU.add, op1=ALU.mult,
                    )
                    nc.vector.tensor_add(
                        _raw(p, s0 - lo, [[m, BL], [1, mq]]),
                        _raw(p, s0 - lo, [[m, BL], [1, mq]]),
                        _raw(q, s0 - lo, [[m, BL], [1, mq]]),
                    )
                r_t = tmp.tile([HID, NBH], F32, tag="r_t")
                nc.vector.scalar_tensor_tensor(
                    _raw(r_t, 0, [[1, n]]),
                    _raw(th, 2 * HID, [[1, n]]),
                    1.0, _raw(th, 3 * HID, [[1, n]]), op0=ALU.add, op1=ALU.mult,
                )
                c_new = state.tile([HID, NBH], F32, tag="c")
                nc.vector.scalar_tensor_tensor(
                    B3(c_new, lo, hi + 1),
                    _raw(p, 0, [[m, BL], [1, m]]),
                    0.5, _raw(r_t, 0, [[m, BL], [1, m]]),
                    op0=ALU.mult, op1=ALU.add,
                )

                tanc = tmp.tile([HID, NBH], F32, tag="tanc")
                nc.scalar.activation(
                    _raw(tanc, 0, [[m, BL], [1, m]]),
                    B3(c_new, lo, hi + 1), AF.Tanh, scale=0.5,
                )
                h_new = state.tile([HID, NBH], F32, tag="h")
                nc.vector.scalar_tensor_tensor(
                    B3(h_new, lo, hi + 1),
                    _raw(th, 4 * HID, [[m, BL], [1, m]]),
                    1.0, _raw(tanc, 0, [[m, BL], [1, m]]),
                    op0=ALU.add, op1=ALU.mult,
                )

                # seed the virgin row t+1 for the next step (both blocks, h+c)
                if t + 1 <= H - 1:
                    for bb in range(BL):
                        nc.gpsimd.tensor_copy(
                            out=_raw(h_new, bb * H + t + 1, [[1, 1]]),
                            in_=hv[:, t:t + 1],
                        )
                        nc.gpsimd.tensor_copy(
                            out=_raw(c_new, bb * H + t + 1, [[1, 1]]),
                            in_=cv[:, t:t + 1],
                        )

                # write H2 into res along the diagonal w = t - r (gpsimd)
                res_ap = _raw(res, (W - 1) * lo + t, [[H * W, BL], [W - 1, m]])
                nc.gpsimd.tensor_copy(out=res_ap, in_=B3(h_new, lo, hi + 1))

                h_cur = h_new
                c_cur = c_new

                # Late-scan interleave: once an 8-row block's diagonal cells
                # are all written (t = 8j+70), add its residual (fp32r
                # matmul; feed-forward so reduced precision is safe) and DMA
                # it out.
                if t >= 70 and (t - 70) % 8 == 0 and (t - 70) // 8 < 8:
                    j = (t - 70) // 8
                    for b in range(BL):
                        cols = slice(b * H * W + 512 * j, b * H * W + 512 * j + 512)
                        rp = pbank()
                        nc.tensor.matmul(
                            rp, wres, xres[:, cols], start=True, stop=True
                        )
                        nc.vector.tensor_add(res[:, cols], res[:, cols], rp)
                        nc.sync.dma_start(out=out_d[:, cols], in_=res[:, cols])

    nc.finalize()
    return nc


_NC_CACHE = {}


def _get_nc():
    if "nc" not in _NC_CACHE:
        _NC_CACHE["nc"] = build_program()
    return _NC_CACHE["nc"]


def _round_fp32r(x):
    """RNE to fp32r (11 explicit mantissa bits), matching PE input rounding."""
    u = np.ascontiguousarray(x, np.float32).view(np.uint32).astype(np.uint64)
    drop = 12
    u2 = u + ((1 << (drop - 1)) - 1) + ((u >> drop) & 1)
    u2 &= ~np.uint64((1 << drop) - 1)
    return u2.astype(np.uint32).view(np.float32)


def _virgin_tables(w_ss, b_is, b_ss):
    """fp64 recurrence for the shared zero-input state v_t, t = 0..62.

    Rows r > t all hold v_t (their whole dependency cone saw zero x), so the
    device only computes the live band and copies v_t into row t+1.
    Returns device-convention tables (2h, 2c), [HID, 63]."""
    w0 = np.asarray(w_ss, np.float64)[:, :, 0]
    w1 = np.asarray(w_ss, np.float64)[:, :, 1]
    bb = np.asarray(b_is, np.float64) + np.asarray(b_ss, np.float64)
    wsum = w0 + w1
    h = np.zeros(HID)
    c = np.zeros(HID)
    hv = np.zeros((HID, H - 1), np.float64)
    cv = np.zeros((HID, H - 1), np.float64)
    for t in range(H - 1):
        z = bb + wsum @ h
        o, fl, fu, i, g = np.split(z, 5)
        sig = lambda v: 1.0 / (1.0 + np.exp(-v))
        o, fl, fu, i = sig(o), sig(fl), sig(fu), sig(i)
        c = fl * c + fu * c + i * np.tanh(g)
        h = o * np.tanh(c)
        hv[:, t] = 2.0 * h
        cv[:, t] = 2.0 * c
    return hv.astype(np.float32), cv.astype(np.float32)


def _prep_inputs(x, w_is, b_is, w_ss, b_ss, w_res, b_res):
    x = np.asarray(x, np.float32)
    # band-packed skewed x: col _BASE[t] + b*m + (r - lo) = x[b, :, r, t - r]
    xs = x.reshape(NCORES, BL, C, H, W)
    xsk = np.zeros((NCORES, C, XC), np.float32)
    for t in range(SW):
        lo, hi, m = _LO[t], _HI[t], _M[t]
        rows = np.arange(lo, hi + 1)
        blk = xs[:, :, :, rows, t - rows]          # [cores, BL, C, m]
        blk = blk.transpose(0, 2, 1, 3)            # [cores, C, BL, m]
        xsk[:, :, _BASE[t]:_BASE[t + 1]] = blk.reshape(NCORES, C, BL * m)

    xres = np.asarray(x).reshape(NCORES, BL, C, H, W).transpose(0, 2, 1, 3, 4)
    xres = xres.reshape(NCORES, C, NRES)
    xres = np.concatenate([xres, np.ones((NCORES, 1, NRES), np.float32)], axis=1)
    xres = _round_fp32r(xres).reshape(NCORES, C + 1, NRES)

    # gate scaling: chunks 0..3 (o, f_left, f_up, i) are sigmoid gates,
    # computed via tanh(z/2) -> pre-halve their weights and biases.
    gs = np.ones((5 * HID,), np.float32)
    gs[0:4 * HID] = 0.5

    # wtap[i, tap*640 + o] = w_ss[o, i, tap] * gs[o] * 0.5
    # (extra 0.5: the kernel's h state holds 2h)
    wtap = np.asarray(w_ss, np.float32).transpose(1, 2, 0) * (0.5 * gs)[None, None, :]
    wtap = np.ascontiguousarray(wtap.reshape(HID, 2 * 5 * HID), np.float32)
    wis = np.ascontiguousarray(
        np.asarray(w_is, np.float32).T * gs[None, :], np.float32
    )
    # per-partition gate bias, fed through the ACT bias port: [128, 5]
    bvec = (np.asarray(b_is, np.float32) + np.asarray(b_ss, np.float32)) * gs
    biases = np.ascontiguousarray(bvec.reshape(5, HID).T, np.float32)
    # x2: the device residual tile accumulates 2*(residual + sum h); the
    # host halves the final output.
    wres = 2.0 * np.concatenate(
        [np.asarray(w_res, np.float32).T, np.asarray(b_res, np.float32)[None, :]],
        axis=0,
    ).astype(np.float32)
    wres = _round_fp32r(wres).reshape(C + 1, HID)

    hv, cv = _virgin_tables(w_ss, b_is, b_ss)

    in_maps = []
    for c in range(NCORES):
        in_maps.append({
            "xsk": np.ascontiguousarray(xsk[c]),
            "xres": np.ascontiguousarray(xres[c]),
            "wtap": wtap,
            "wis": wis,
            "wres": wres,
            "bias": biases,
            "hv": hv,
            "cv": cv,
        })
    return in_maps


def kernel(x, w_is, b_is, w_ss, b_ss, w_res, b_res, _trace=False):
    nc = _get_nc()
    in_maps = _prep_inputs(x, w_is, b_is, w_ss, b_ss, w_res, b_res)
    r = run_bass_kernel_spmd(nc, in_maps, list(range(NCORES)), trace=_trace)
    outs = [r.results[c]["out"] for c in range(NCORES)]
    out = np.stack(outs, 0).reshape(NCORES, HID, BL, H, W)
    out = out.transpose(0, 2, 1, 3, 4).reshape(B, HID, H, W)
    return np.ascontiguousarray(out * np.float32(0.5))


# revision 12
# speedup vs baseline: 1.2692x; 1.0261x over previous
"""DiagonalLSTM Trainium2 kernel — band-restricted scan.

Sharding: data-parallel over batch B=16 across 8 cores (2 batch elems/core).
Per-core layout: partitions = 128-wide HID gate chunks, free dim = (b, j)
where j indexes the LIVE DIAGONAL BAND rows [lo..hi], lo = max(0, t-63),
hi = min(t, 63).

Key reduction vs the full-width scan: rows r > t ("virgin" rows, zero x so
far) all share one state vector v_t that depends only on t, so they are not
computed on-device at all.  A host-precomputed fp64 table of v_t (h and c,
device 2x convention) seeds row t+1 each step via two 1-col gpsimd copies.
Per-step matmul/ACT/DVE free size drops from avg 96 to avg 64.5 columns and
the x-side input is pre-packed band-only (xsk [64, 8192] vs [64, 16256]).

Per scan step t (127 steps), each of the 5 gate chunks accumulates in PSUM:
    wis_chunk @ x_band   (K=64, packed band cols, opens the group)
  + w0_chunk @ h_prev    row-shifted (skipped at t=0)
  + w1_chunk @ h_prev
All scan matmuls fp32: the scan dynamics chaotically amplify per-step input
rounding (measured: fp32r inputs -> rel err 1.4), so only the feed-forward
residual matmul uses fp32r (4x faster, error enters once, ~1e-4).

Sigmoid gates computed as 0.5*(1+tanh(x/2)) via pre-halved weights; ONE tanh
activation per gate chunk (fires as soon as its PSUM bank closes, keeping the
serial chain short).  State convention: h_cur holds 2h, c_cur holds 2c; res
accumulates 2*(h+residual) and the host halves the output.
"""

import numpy as np

import concourse.bass as bass
import concourse.mybir as mybir
from concourse import bacc
from concourse import tile
from concourse.bass_utils import run_bass_kernel_spmd

B, C, H, W = 16, 64, 64, 64
HID = 128
SW = H + W - 1  # 127
NCORES = 8
BL = B // NCORES  # 2
NBH = BL * H       # 128 state cols (b, r)
NRES = BL * H * W  # 8192 output cols

F32 = mybir.dt.float32
F32R = mybir.dt.float32r
AF = mybir.ActivationFunctionType
ALU = mybir.AluOpType

# band geometry per step (shared host/device)
_LO = [max(0, t - (W - 1)) for t in range(SW)]
_HI = [min(t, H - 1) for t in range(SW)]
_M = [hi - lo + 1 for lo, hi in zip(_LO, _HI)]
_BASE = np.concatenate([[0], np.cumsum([BL * m for m in _M])]).astype(int)
XC = int(_BASE[-1])  # 8192

# scan chunk emission order (gate chunk index k): fl, fu, i, g, o
KORD = (1, 2, 3, 4, 0)


def _raw(t, off, dims):
    """Raw AP on tile t: keep its partition pair, custom free dims."""
    return bass.AP(t.tensor, t.offset + off, [list(t.ap[0])] + [list(d) for d in dims])


def build_program():
    nc = bacc.Bacc(None, target_bir_lowering=False)

    xsk_d = nc.dram_tensor("xsk", [C + 1, XC], F32, kind="ExternalInput")
    xres_d = nc.dram_tensor("xres", [C + 1, NRES], F32R, kind="ExternalInput")
    wtap_d = nc.dram_tensor("wtap", [HID, 2 * 5 * HID], F32, kind="ExternalInput")
    wis_d = nc.dram_tensor("wis", [C + 1, 5 * HID], F32, kind="ExternalInput")
    wres_d = nc.dram_tensor("wres", [C + 1, HID], F32R, kind="ExternalInput")
    hv_d = nc.dram_tensor("hv", [HID, H - 1], F32, kind="ExternalInput")
    cv_d = nc.dram_tensor("cv", [HID, H - 1], F32, kind="ExternalInput")
    out_d = nc.dram_tensor("out", [HID, NRES], F32, kind="ExternalOutput")

    with tile.TileContext(nc) as tc:
        with (
            tc.tile_pool(name="const", bufs=1) as const,
            tc.tile_pool(name="state", bufs=3) as state,
            tc.tile_pool(name="tmp", bufs=3) as tmp,
            tc.tile_pool(name="gpsumA", bufs=3, space="PSUM") as gpsumA,
            tc.tile_pool(name="gpsumO", bufs=2, space="PSUM") as gpsumO,
        ):
            xsk = const.tile([C + 1, XC], F32)
            xres = const.tile([C + 1, NRES], F32R)
            wtap = const.tile([HID, 2 * 5 * HID], F32)
            wis = const.tile([C + 1, 5 * HID], F32)
            wres = const.tile([C + 1, HID], F32R)
            hv = const.tile([HID, H - 1], F32)
            cv = const.tile([HID, H - 1], F32)
            res = const.tile([HID, NRES], F32)

            nc.sync.dma_start(out=wis, in_=wis_d[:])
            nc.sync.dma_start(out=wtap, in_=wtap_d[:])
            nc.sync.dma_start(out=hv, in_=hv_d[:])
            nc.sync.dma_start(out=cv, in_=cv_d[:])
            nc.sync.dma_start(out=wres, in_=wres_d[:])
            steps_cut = [0, 4, 12, 24, 36, 48, 64, 80, 100, SW]
            for a, b in zip(steps_cut[:-1], steps_cut[1:]):
                lo_e, hi_e = int(_BASE[a]), int(_BASE[b])
                nc.sync.dma_start(out=xsk[:, lo_e:hi_e], in_=xsk_d[:, lo_e:hi_e])
            nc.sync.dma_start(out=xres, in_=xres_d[:])

            def pbankA():
                ps = gpsumA.tile([HID, 1024], F32, tag="A")
                return ps

            def pbankO():
                ps = gpsumO.tile([HID, 512], F32, tag="O")
                return ps

            # ---- scan state: (b, r) layout [HID, 128] ----
            h_cur = state.tile([HID, NBH], F32, tag="h")
            c_cur = state.tile([HID, NBH], F32, tag="c")
            nc.vector.memzero(h_cur)
            nc.vector.memzero(c_cur)

            def B3(ap, a, b):
                """(b, r) state view, rows [a..b) of each batch block."""
                return ap.rearrange("p (b r) -> p b r", b=BL)[:, :, a:b]

            # gate chunk placement: A0 = {fl, fu}, A1 = {i, g}, O = {o};
            # each chunk in its own PSUM bank (512-col offsets), pairs share
            # a 2-bank tile so ONE activation covers both chunks.
            def slots(tiles):
                a0, a1, po = tiles
                return ((a0, 0), (a0, 512), (a1, 0), (a1, 512), (po, 0))

            def xmm(t):
                """i_s matmuls for step t: packed band cols (opens groups).
                K=65: the ones row of xsk adds the per-gate bias."""
                b0, n = int(_BASE[t]), BL * _M[t]
                tiles = (pbankA(), pbankA(), pbankO())
                for idx, k in enumerate(KORD):
                    pk, off = slots(tiles)[idx]
                    nc.tensor.matmul(
                        _raw(pk, off, [[1, n]]),
                        wis[:, k * HID:(k + 1) * HID],
                        xsk[:, b0:b0 + n],
                        start=True, stop=False,
                    )
                return tiles

            pcur = xmm(0)

            for t in range(SW):
                lo, hi, m = _LO[t], _HI[t], _M[t]
                n = BL * m
                s0 = max(lo, 1)
                mq = hi - s0 + 1  # rows with a defined (r-1) neighbor

                th = tmp.tile([HID, 5 * HID], F32, tag="th")
                for idx, k in enumerate(KORD):
                    pk, off = slots(pcur)[idx]
                    w0c = wtap[:, k * HID:(k + 1) * HID]
                    w1c = wtap[:, 5 * HID + k * HID:5 * HID + (k + 1) * HID]
                    if mq > 0:
                        nc.tensor.matmul(
                            _raw(pk, off + s0 - lo, [[m, BL], [1, mq]]),
                            w0c,
                            B3(h_cur, s0 - 1, hi),
                            start=False, stop=False,
                        )
                    nc.tensor.matmul(
                        _raw(pk, off, [[m, BL], [1, m]]),
                        w1c,
                        B3(h_cur, lo, hi + 1),
                        start=False, stop=True,
                    )
                    # paired tanh: fires when both banks of the pair close
                    if idx in (1, 3):
                        nc.scalar.activation(
                            _raw(th, (idx - 1) * HID, [[HID, 2], [1, n]]),
                            _raw(pk, 0, [[512, 2], [1, n]]),
                            AF.Tanh,
                        )
                    elif idx == 4:
                        nc.scalar.activation(
                            _raw(th, 4 * HID, [[1, n]]),
                            _raw(pk, 0, [[1, n]]),
                            AF.Tanh,
                        )

                # prefetch next step's x-side matmuls while ACT/DVE run
                if t + 1 < SW:
                    pcur = xmm(t + 1)

                # P = (t_fl+1)*C2 ; P += (t_fu+1)*C2sh (rows >= s0);
                # C2' = 0.5*P + (t_i+1)*g  on band rows
                p = tmp.tile([HID, NBH], F32, tag="p")
                nc.vector.scalar_tensor_tensor(
                    _raw(p, 0, [[m, BL], [1, m]]),
                    _raw(th, 0 * HID, [[m, BL], [1, m]]),
                    1.0, B3(c_cur, lo, hi + 1), op0=ALU.add, op1=ALU.mult,
                )
                if mq > 0:
                    q = tmp.tile([HID, NBH], F32, tag="q")
                    nc.vector.scalar_tensor_tensor(
                        _raw(q, s0 - lo, [[m, BL], [1, mq]]),
                        _raw(th, 1 * HID + (s0 - lo), [[m, BL], [1, mq]]),
                        1.0, B3(c_cur, s0 - 1, hi), op0=ALU.add, op1=ALU.mult,
                    )
                    nc.vector.tensor_add(
                        _raw(p, s0 - lo, [[m, BL], [1, mq]]),
                        _raw(p, s0 - lo, [[m, BL], [1, mq]]),
                        _raw(q, s0 - lo, [[m, BL], [1, mq]]),
                    )
                r_t = tmp.tile([HID, NBH], F32, tag="r_t")
                nc.vector.scalar_tensor_tensor(
                    _raw(r_t, 0, [[1, n]]),
                    _raw(th, 2 * HID, [[1, n]]),
                    1.0, _raw(th, 3 * HID, [[1, n]]), op0=ALU.add, op1=ALU.mult,
                )
                c_new = state.tile([HID, NBH], F32, tag="c")
                nc.vector.scalar_tensor_tensor(
                    B3(c_new, lo, hi + 1),
                    _raw(p, 0, [[m, BL], [1, m]]),
                    0.5, _raw(r_t, 0, [[m, BL], [1, m]]),
                    op0=ALU.mult, op1=ALU.add,
                )

                tanc = tmp.tile([HID, NBH], F32, tag="tanc")
                nc.scalar.activation(
                    _raw(tanc, 0, [[m, BL], [1, m]]),
                    B3(c_new, lo, hi + 1), AF.Tanh, scale=0.5,
                )
                h_new = state.tile([HID, NBH], F32, tag="h")
                nc.vector.scalar_tensor_tensor(
                    B3(h_new, lo, hi + 1),
                    _raw(th, 4 * HID, [[m, BL], [1, m]]),
                    1.0, _raw(tanc, 0, [[m, BL], [1, m]]),
                    op0=ALU.add, op1=ALU.mult,
                )

                # seed the virgin row t+1 for the next step (both blocks, h+c)
                if t + 1 <= H - 1:
                    for bb in range(BL):
                        nc.gpsimd.tensor_copy(
                            out=_raw(h_new, bb * H + t + 1, [[1, 1]]),
                            in_=hv[:, t:t + 1],
                        )
                        nc.gpsimd.tensor_copy(
                            out=_raw(c_new, bb * H + t + 1, [[1, 1]]),
                            in_=cv[:, t:t + 1],
                        )

                # write H2 into res along the diagonal w = t - r (gpsimd)
                res_ap = _raw(res, (W - 1) * lo + t, [[H * W, BL], [W - 1, m]])
                nc.gpsimd.tensor_copy(out=res_ap, in_=B3(h_new, lo, hi + 1))

                h_cur = h_new
                c_cur = c_new

                # Late-scan interleave: once an 8-row block's diagonal cells
                # are all written (t = 8j+70), add its residual (fp32r
                # matmul; feed-forward so reduced precision is safe) and DMA
                # it out.
                if t >= 70 and (t - 70) % 8 == 0 and (t - 70) // 8 < 8:
                    j = (t - 70) // 8
                    for b in range(BL):
                        cols = slice(b * H * W + 512 * j, b * H * W + 512 * j + 512)
                        rp = pbankO()
                        nc.tensor.matmul(
                            rp, wres, xres[:, cols], start=True, stop=True
                        )
                        nc.vector.tensor_add(res[:, cols], res[:, cols], rp)
                        nc.sync.dma_start(out=out_d[:, cols], in_=res[:, cols])

    nc.finalize()
    return nc


_NC_CACHE = {}


def _get_nc():
    if "nc" not in _NC_CACHE:
        _NC_CACHE["nc"] = build_program()
    return _NC_CACHE["nc"]


def _round_fp32r(x):
    """RNE to fp32r (11 explicit mantissa bits), matching PE input rounding."""
    u = np.ascontiguousarray(x, np.float32).view(np.uint32).astype(np.uint64)
    drop = 12
    u2 = u + ((1 << (drop - 1)) - 1) + ((u >> drop) & 1)
    u2 &= ~np.uint64((1 << drop) - 1)
    return u2.astype(np.uint32).view(np.float32)


def _virgin_tables(w_ss, b_is, b_ss):
    """fp64 recurrence for the shared zero-input state v_t, t = 0..62.

    Rows r > t all hold v_t (their whole dependency cone saw zero x), so the
    device only computes the live band and copies v_t into row t+1.
    Returns device-convention tables (2h, 2c), [HID, 63]."""
    w0 = np.asarray(w_ss, np.float64)[:, :, 0]
    w1 = np.asarray(w_ss, np.float64)[:, :, 1]
    bb = np.asarray(b_is, np.float64) + np.asarray(b_ss, np.float64)
    wsum = w0 + w1
    h = np.zeros(HID)
    c = np.zeros(HID)
    hv = np.zeros((HID, H - 1), np.float64)
    cv = np.zeros((HID, H - 1), np.float64)
    for t in range(H - 1):
        z = bb + wsum @ h
        o, fl, fu, i, g = np.split(z, 5)
        sig = lambda v: 1.0 / (1.0 + np.exp(-v))
        o, fl, fu, i = sig(o), sig(fl), sig(fu), sig(i)
        c = fl * c + fu * c + i * np.tanh(g)
        h = o * np.tanh(c)
        hv[:, t] = 2.0 * h
        cv[:, t] = 2.0 * c
    return hv.astype(np.float32), cv.astype(np.float32)


def _prep_inputs(x, w_is, b_is, w_ss, b_ss, w_res, b_res):
    x = np.asarray(x, np.float32)
    # band-packed skewed x: col _BASE[t] + b*m + (r - lo) = x[b, :, r, t - r]
    xs = x.reshape(NCORES, BL, C, H, W)
    xsk = np.zeros((NCORES, C + 1, XC), np.float32)
    xsk[:, C, :] = 1.0  # ones row: adds the per-gate bias via the matmul
    for t in range(SW):
        lo, hi, m = _LO[t], _HI[t], _M[t]
        rows = np.arange(lo, hi + 1)
        blk = xs[:, :, :, rows, t - rows]          # [cores, BL, C, m]
        blk = blk.transpose(0, 2, 1, 3)            # [cores, C, BL, m]
        xsk[:, :C, _BASE[t]:_BASE[t + 1]] = blk.reshape(NCORES, C, BL * m)

    xres = np.asarray(x).reshape(NCORES, BL, C, H, W).transpose(0, 2, 1, 3, 4)
    xres = xres.reshape(NCORES, C, NRES)
    xres = np.concatenate([xres, np.ones((NCORES, 1, NRES), np.float32)], axis=1)
    xres = _round_fp32r(xres).reshape(NCORES, C + 1, NRES)

    # gate scaling: chunks 0..3 (o, f_left, f_up, i) are sigmoid gates,
    # computed via tanh(z/2) -> pre-halve their weights and biases.
    gs = np.ones((5 * HID,), np.float32)
    gs[0:4 * HID] = 0.5

    # wtap[i, tap*640 + o] = w_ss[o, i, tap] * gs[o] * 0.5
    # (extra 0.5: the kernel's h state holds 2h)
    wtap = np.asarray(w_ss, np.float32).transpose(1, 2, 0) * (0.5 * gs)[None, None, :]
    wtap = np.ascontiguousarray(wtap.reshape(HID, 2 * 5 * HID), np.float32)
    # wis row C (the ones row's partner) carries the combined gate bias
    bvec = (np.asarray(b_is, np.float32) + np.asarray(b_ss, np.float32)) * gs
    wis = np.ascontiguousarray(
        np.concatenate(
            [np.asarray(w_is, np.float32).T * gs[None, :], bvec[None, :]], axis=0
        ),
        np.float32,
    )
    # x2: the device residual tile accumulates 2*(residual + sum h); the
    # host halves the final output.
    wres = 2.0 * np.concatenate(
        [np.asarray(w_res, np.float32).T, np.asarray(b_res, np.float32)[None, :]],
        axis=0,
    ).astype(np.float32)
    wres = _round_fp32r(wres).reshape(C + 1, HID)

    hv, cv = _virgin_tables(w_ss, b_is, b_ss)

    in_maps = []
    for c in range(NCORES):
        in_maps.append({
            "xsk": np.ascontiguousarray(xsk[c]),
            "xres": np.ascontiguousarray(xres[c]),
            "wtap": wtap,
            "wis": wis,
            "wres": wres,
            "hv": hv,
            "cv": cv,
        })
    return in_maps


def kernel(x, w_is, b_is, w_ss, b_ss, w_res, b_res, _trace=False):
    nc = _get_nc()
    in_maps = _prep_inputs(x, w_is, b_is, w_ss, b_ss, w_res, b_res)
    r = run_bass_kernel_spmd(nc, in_maps, list(range(NCORES)), trace=_trace)
    outs = [r.results[c]["out"] for c in range(NCORES)]
    out = np.stack(outs, 0).reshape(NCORES, HID, BL, H, W)
    out = out.transpose(0, 2, 1, 3, 4).reshape(B, HID, H, W)
    return np.ascontiguousarray(out * np.float32(0.5))
